# revision 5
# baseline (speedup 1.0000x reference)
"""Performer linear attention (nn_PerformerLinearAttention) — Trainium2 Bass kernel.

Sharding: 8 cores = (batch 2) x (sequence 4); core c handles batch c//4,
positions [(c%4)*1024, (c%4+1)*1024). Chunked causal linear attention with
chunk size 128; cross-core KV/z prefix state via an AllGather (groups
[[0..3],[4..7]]) of per-core KV totals, combined on-device with per-core
prefix-weight inputs (rank-agnostic program).

Phase order (program order ~ Tile priority):
  0 consts/hT/wv loads          1 v projection (pos-major)
  2 k path: proj+rope -> krot (resident), kpT (transient, for ksum),
    kp_pos, KV chains + snapshots, cc_in DMAs
  3 ksum scan, z, AllGather (fires early; later phases overlap it)
  4 q path: proj+rope, qpT (partition-stacked pairs), qsum rows
  5 attention: kpT recompute from krot, AT (masked), numT partial evicts
  6 prefix assembly, denominators, pass 2 (global KV term + normalize)
  7 output projection
"""
import numpy as np

import concourse.bacc as bacc
import concourse.mybir as mybir
import concourse.tile as tile
from concourse import bass_utils

FP32 = mybir.dt.float32
ADD = mybir.AluOpType.add
MULT = mybir.AluOpType.mult
MAX = mybir.AluOpType.max
BYPASS = mybir.AluOpType.bypass

NH, NKV, HD, NF = 16, 8, 64, 64
EPS_K, EPS_D = 1e-4, 1e-6
B, S, HM = 2, 4096, 1024
NCORES, GROUP = 8, 4
NPOS = S // GROUP            # 1024 positions per core
CH = 128
NCH = NPOS // CH             # 8 chunks
NPAIR = NH // 2              # 8 GQA pairs == kv heads


def build_nc():
    nc = bacc.Bacc("TRN2", target_bir_lowering=False, debug=False, num_devices=NCORES)

    # per-core inputs
    hT_d = nc.dram_tensor("hT", [HM, NPOS], FP32, kind="ExternalInput")
    cosT_d = nc.dram_tensor("cosT", [128, NPOS], FP32, kind="ExternalInput")
    sinTs_d = nc.dram_tensor("sinTs", [128, NPOS], FP32, kind="ExternalInput")
    wpref_d = nc.dram_tensor("wpref", [128, GROUP], FP32, kind="ExternalInput")
    # shared inputs
    wqkT_d = nc.dram_tensor("wqkT", [HM, 1536], FP32, kind="ExternalInput")
    wvT_d = nc.dram_tensor("wvT", [HM, 512], FP32, kind="ExternalInput")
    woT_d = nc.dram_tensor("woT", [HM, HM], FP32, kind="ExternalInput")
    projS2_d = nc.dram_tensor("projS2", [128, NF], FP32, kind="ExternalInput")
    permP_d = nc.dram_tensor("permP", [128, 128], FP32, kind="ExternalInput")
    tri_d = nc.dram_tensor("tri", [128, 128], FP32, kind="ExternalInput")
    ones128_d = nc.dram_tensor("ones128", [128, 1], FP32, kind="ExternalInput")
    onehot8_d = nc.dram_tensor("onehot8", [128, 512], FP32, kind="ExternalInput")
    # output
    out_d = nc.dram_tensor("outp", [NPOS, HM], FP32, kind="ExternalOutput")

    with tile.TileContext(nc) as tc:
        with (
            tc.tile_pool(name="consts", bufs=1) as cp,
            tc.tile_pool(name="main", bufs=1) as mp,
            tc.tile_pool(name="dram", bufs=1, space="DRAM") as dp,
            tc.tile_pool(name="psA", bufs=2, space="PSUM") as psA,
            tc.tile_pool(name="psB", bufs=4, space="PSUM") as psB,
            tc.tile_pool(name="psKV", bufs=2, space="PSUM") as psKV,
        ):
            # ---- constants ----
            cosT = cp.tile([128, NPOS], FP32, tag="cosT")
            nc.sync.dma_start(cosT[:], cosT_d[:])
            sinTs = cp.tile([128, NPOS], FP32, tag="sinTs")
            nc.sync.dma_start(sinTs[:], sinTs_d[:])
            projS2 = cp.tile([128, NF], FP32, tag="projS2")
            nc.sync.dma_start(projS2[:], projS2_d[:])
            permP = cp.tile([128, 128], FP32, tag="permP")
            nc.sync.dma_start(permP[:], permP_d[:])
            tri = cp.tile([128, 128], FP32, tag="tri")
            nc.sync.dma_start(tri[:], tri_d[:])
            ones128 = cp.tile([128, 1], FP32, tag="ones128")
            nc.sync.dma_start(ones128[:], ones128_d[:])
            onehot8 = cp.tile([128, 512], FP32, tag="onehot8")
            nc.sync.dma_start(onehot8[:], onehot8_d[:])
            wpref = cp.tile([128, GROUP], FP32, tag="wpref")
            nc.sync.dma_start(wpref[:], wpref_d[:])

            # ---- persistent (whole-kernel) arrays ----
            qpT = [mp.tile([128, NPOS], FP32, tag=f"qpT{p}", name=f"qpT{p}")
                   for p in range(NPAIR)]
            v_sb = [mp.tile([128, 512], FP32, tag=f"v{c}", name=f"v{c}")
                    for c in range(NCH)]
            krot = [mp.tile([128, NPOS], FP32, tag=f"krot{i}", name=f"krot{i}")
                    for i in range(4)]
            snaps = [[mp.tile([128, 64], FP32, tag=f"snap{g}_{c}",
                              name=f"snap{g}_{c}") for c in range(NCH)]
                     for g in range(NKV)]
            kz8 = mp.tile([8, NPOS], FP32, tag="kz8")    # ksum -> ztot (in-place)
            qre = mp.tile([8, NPOS], FP32, tag="qre")    # qsum_e -> r_e (in-place)
            qro = mp.tile([8, NPOS], FP32, tag="qro")    # qsum_o -> r_o (in-place)
            KVg = [mp.tile([128, 64], FP32, tag=f"kvg{g}", name=f"kvg{g}")
                   for g in range(NKV)]
            zgt = mp.tile([8, GROUP], FP32, tag="zgt")
            zpref = mp.tile([8, 1], FP32, tag="zpref")

            cc_in = dp.tile([520, 64], FP32, tag="cc_in")
            cc_out = dp.tile([GROUP * 520, 64], FP32, tag="cc_out")

            with (
                tc.tile_pool(name="phA", bufs=1) as pA,
                tc.tile_pool(name="trA", bufs=2) as tA,
                tc.tile_pool(name="wqks", bufs=4) as wqp,
            ):
                hTs = []
                for m in range(8):
                    t = pA.tile([128, NPOS], FP32, tag=f"hT{m}", name=f"hT{m}")
                    nc.sync.dma_start(t[:], hT_d[m * 128:(m + 1) * 128, :])
                    hTs.append(t)

                def proj_pair(rb, dest):
                    """QKV projection + RoPE for row-block rb (one head pair).
                    Writes the rope'd pair into `dest` [128, 1024]."""
                    for half in range(2):
                        ps = psA.tile([128, 512], FP32, tag="acc")
                        for m in range(8):
                            wq = wqp.tile([128, 128], FP32, tag="wqk")
                            nc.sync.dma_start(
                                wq[:], wqkT_d[m * 128:(m + 1) * 128,
                                              rb * 128:(rb + 1) * 128])
                            nc.tensor.matmul(ps[:], wq[:],
                                             hTs[m][:, half * 512:(half + 1) * 512],
                                             start=(m == 0), stop=(m == 7))
                        nc.scalar.copy(dest[:, half * 512:(half + 1) * 512], ps[:])
                    for half in range(2):
                        hs = slice(half * 512, (half + 1) * 512)
                        rps = psB.tile([128, 512], FP32, tag="ps")
                        nc.tensor.matmul(rps[:], permP[:], dest[:, hs],
                                         start=True, stop=True)
                        tmp = tA.tile([128, 512], FP32, tag="ropetmp")
                        nc.vector.tensor_tensor(out=tmp[:], in0=dest[:, hs],
                                                in1=cosT[:, hs], op=MULT)
                        nc.vector.tensor_tensor(out=rps[:], in0=rps[:],
                                                in1=sinTs[:, hs], op=MULT)
                        nc.vector.tensor_tensor(out=dest[:, hs], in0=tmp[:],
                                                in1=rps[:], op=ADD)

                def row_sum(dst_row_ap, rhs_ap, base):
                    """dst_row_ap [1, 512] (DRAM-side of a SBUF row via DMA) =
                    column sums of rhs_ap [64, 512] at partition base `base`."""
                    sps = psB.tile([1, 512], FP32, tag="ps")
                    nc.tensor.matmul(sps[:], ones128[base:base + 64, :], rhs_ap,
                                     start=True, stop=True)
                    stage = tA.tile([1, 512], FP32, tag="rowstage")
                    nc.scalar.copy(stage[:], sps[:])
                    nc.sync.dma_start(dst_row_ap, stage[:])

                # ---------- phase 1: v projection ----------
                with tc.tile_pool(name="phWV", bufs=1) as pwv:
                    wv = []
                    for m in range(8):
                        t = pwv.tile([128, 512], FP32, tag=f"wv{m}", name=f"wv{m}")
                        nc.sync.dma_start(t[:], wvT_d[m * 128:(m + 1) * 128, :])
                        wv.append(t)
                    for c in range(NCH):
                        cs = slice(c * 128, (c + 1) * 128)
                        ps = psA.tile([128, 512], FP32, tag="acc")
                        for m in range(8):
                            nc.tensor.matmul(ps[:], hTs[m][:, cs], wv[m][:],
                                             start=(m == 0), stop=(m == 7))
                        nc.scalar.copy(v_sb[c][:], ps[:])

                # ---------- phase 2: k path ----------
                for i in range(4):          # rb = 8 + i ; kv heads 2i, 2i+1
                    proj_pair(8 + i, krot[i])
                    for hh in range(2):
                        g = 2 * i + hh
                        base = hh * 64
                        hsl = slice(base, base + 64)
                        # kpT (f-major) for ksum only — transient
                        kpt = tA.tile([64, NPOS], FP32, tag="kpt_tmp")
                        for half in range(2):
                            hs = slice(half * 512, (half + 1) * 512)
                            fps = psB.tile([64, 512], FP32, tag="ps")
                            nc.tensor.matmul(fps[:], projS2[hsl, :],
                                             krot[i][hsl, hs],
                                             start=True, stop=True)
                            nc.vector.tensor_scalar_max(kpt[:, hs], fps[:], 0.0)
                            row_sum(kz8[g:g + 1, hs], kpt[:, hs], 0)
                        # kp_pos + KV chain
                        kv_ps = psKV.tile([64, 64], FP32, tag="kv")
                        for c in range(NCH):
                            cs = slice(c * 128, (c + 1) * 128)
                            pps = psB.tile([128, 64], FP32, tag="ps")
                            nc.tensor.matmul(pps[:], krot[i][hsl, cs],
                                             projS2[hsl, :], start=True, stop=True)
                            kp_sb = tA.tile([128, 64], FP32, tag="kpos")
                            nc.vector.tensor_scalar_max(kp_sb[:], pps[:], 0.0)
                            nc.tensor.matmul(kv_ps[:], kp_sb[:],
                                             v_sb[c][:, g * 64:(g + 1) * 64],
                                             start=(c == 0), stop=(c == NCH - 1))
                            nc.scalar.copy(snaps[g][c][0:64, :], kv_ps[:])
                            nc.scalar.copy(snaps[g][c][64:128, :], kv_ps[:])
                        nc.sync.dma_start(cc_in[g * 64:(g + 1) * 64, :],
                                          snaps[g][NCH - 1][0:64, :])

                # ---------- phase 3: scan + collective ----------
                nc.vector.tensor_tensor_scan(
                    kz8[:, 0:512], kz8[:, 0:512], kz8[:, 0:512],
                    0.0, op0=ADD, op1=BYPASS)
                nc.vector.tensor_tensor_scan(
                    kz8[:, 512:1024], kz8[:, 512:1024], kz8[:, 512:1024],
                    kz8[:, 511:512], op0=ADD, op1=BYPASS)
                ztile = tA.tile([8, 1], FP32, tag="ztile")
                nc.vector.tensor_copy(ztile[:], kz8[:, NPOS - 1:NPOS])
                nc.sync.dma_start(cc_in[512:520, 0:1], ztile[:])
                nc.gpsimd.collective_compute(
                    "AllGather", BYPASS,
                    ins=[cc_in[:].opt()], outs=[cc_out[:].opt()],
                    replica_groups=[[0, 1, 2, 3], [4, 5, 6, 7]])

                # ---------- phase 4: q path ----------
                for g in range(NPAIR):
                    proj_pair(g, qpT[g])     # qpT[g] briefly holds rope'd q pair
                    for hh in range(2):
                        base = hh * 64
                        hsl = slice(base, base + 64)
                        for half in range(2):
                            hs = slice(half * 512, (half + 1) * 512)
                            fps = psB.tile([64, 512], FP32, tag="ps")
                            nc.tensor.matmul(fps[:], projS2[hsl, :], qpT[g][hsl, hs],
                                             start=True, stop=True)
                            nc.vector.tensor_scalar(qpT[g][hsl, hs], fps[:], 0.0,
                                                    EPS_K, op0=MAX, op1=ADD)
                    for hh in range(2):
                        base = hh * 64
                        qdst = qre if hh == 0 else qro
                        for half in range(2):
                            hs = slice(half * 512, (half + 1) * 512)
                            row_sum(qdst[g:g + 1, hs],
                                    qpT[g][base:base + 64, hs], base)

            # ---------- phases 5-7 ----------
            with (
                tc.tile_pool(name="phB", bufs=1) as pB,
                tc.tile_pool(name="trB", bufs=3) as tB,
            ):
                attnT = [pB.tile([128, NPOS], FP32, tag=f"attnT{p}",
                                 name=f"attnT{p}") for p in range(NPAIR)]

                # phase 5: attention (kpT recompute, AT, numerator partial)
                for g in range(NKV):
                    i, hh = g // 2, g % 2
                    base = hh * 64
                    hsl = slice(base, base + 64)
                    kpt = tB.tile([128, NPOS], FP32, tag="kpt2", bufs=2)
                    for half in range(2):
                        hs = slice(half * 512, (half + 1) * 512)
                        fps = psB.tile([64, 512], FP32, tag="ps")
                        nc.tensor.matmul(fps[:], projS2[hsl, :], krot[i][hsl, hs],
                                         start=True, stop=True)
                        nc.vector.tensor_scalar_max(kpt[0:64, hs], fps[:], 0.0)
                        nc.vector.tensor_scalar_max(kpt[64:128, hs], fps[:], 0.0)
                    for c in range(NCH):
                        cs = slice(c * 128, (c + 1) * 128)
                        for hh2 in range(2):
                            b2 = hh2 * 64
                            h2sl = slice(b2, b2 + 64)
                            at = psB.tile([128, 128], FP32, tag="ps")
                            nc.tensor.matmul(at[:], kpt[h2sl, cs],
                                             qpT[g][h2sl, cs], start=True, stop=True)
                            ATm = tB.tile([128, 128], FP32, tag="atm")
                            nc.vector.tensor_tensor(out=ATm[:], in0=at[:],
                                                    in1=tri[:], op=MULT)
                            nps = psB.tile([64, 128], FP32, tag="ps")
                            nc.tensor.matmul(nps[:], v_sb[c][:, g * 64:(g + 1) * 64],
                                             ATm[:], start=True, stop=(c == 0))
                            if c > 0:
                                nc.tensor.matmul(nps[:], snaps[g][c - 1][h2sl, :],
                                                 qpT[g][h2sl, cs],
                                                 start=False, stop=True)
                            nc.scalar.copy(attnT[g][b2:b2 + 64, cs], nps[:])

                # phase 6a: prefix assembly
                for g in range(NKV):
                    for rho in range(GROUP):
                        gt = tB.tile([128, 64], FP32, tag="gath")
                        src = cc_out[rho * 520 + g * 64:rho * 520 + (g + 1) * 64, :]
                        nc.sync.dma_start(gt[0:64, :], src)
                        nc.sync.dma_start(gt[64:128, :], src)
                        if rho == 0:
                            nc.vector.tensor_scalar_mul(KVg[g][:], gt[:],
                                                        wpref[:, 0:1])
                        else:
                            nc.vector.scalar_tensor_tensor(
                                out=KVg[g][:], in0=gt[:],
                                scalar=wpref[:, rho:rho + 1],
                                in1=KVg[g][:], op0=MULT, op1=ADD)
                for rho in range(GROUP):
                    nc.sync.dma_start(zgt[:, rho:rho + 1],
                                      cc_out[rho * 520 + 512:rho * 520 + 520, 0:1])
                nc.vector.tensor_scalar_mul(zpref[:], zgt[:, 0:1], wpref[0:8, 0:1])
                for rho in range(1, GROUP):
                    nc.vector.scalar_tensor_tensor(
                        out=zpref[:], in0=zgt[:, rho:rho + 1],
                        scalar=wpref[0:8, rho:rho + 1], in1=zpref[:],
                        op0=MULT, op1=ADD)
                # phase 6b: denominators (in place)
                nc.vector.tensor_scalar_add(kz8[:], kz8[:], zpref[:, 0:1])
                for qt in (qre, qro):
                    nc.vector.tensor_tensor(out=qt[:], in0=qt[:], in1=kz8[:],
                                            op=MULT)
                    nc.vector.tensor_scalar_add(qt[:], qt[:], EPS_D)
                    nc.vector.reciprocal(qt[:], qt[:])

                # phase 6c: pass 2 — global KV term + normalize
                for g in range(NKV):
                    for hh in range(2):
                        base = hh * 64
                        hsl = slice(base, base + 64)
                        rtile = qre if hh == 0 else qro
                        for half in range(2):
                            hs = slice(half * 512, (half + 1) * 512)
                            kvp = psB.tile([64, 512], FP32, tag="ps")
                            nc.tensor.matmul(kvp[:], KVg[g][hsl, :], qpT[g][hsl, hs],
                                             start=True, stop=True)
                            bps = psB.tile([64, 512], FP32, tag="ps")
                            nc.tensor.matmul(bps[:],
                                             onehot8[0:8, g * 64:(g + 1) * 64],
                                             rtile[:, hs],
                                             start=True, stop=True)
                            dst = attnT[g][hsl, hs]
                            nc.vector.tensor_tensor(out=dst, in0=dst, in1=kvp[:],
                                                    op=ADD)
                            nc.vector.tensor_tensor(out=dst, in0=dst, in1=bps[:],
                                                    op=MULT)

                # phase 7: output projection
                wo = []
                for t in range(8):
                    w = pB.tile([128, NPOS], FP32, tag=f"wo{t}", name=f"wo{t}")
                    nc.sync.dma_start(w[:], woT_d[t * 128:(t + 1) * 128, :])
                    wo.append(w)
                for c in range(NCH):
                    cs = slice(c * 128, (c + 1) * 128)
                    for mh in range(2):
                        ops_ = psA.tile([128, 512], FP32, tag="acc")
                        for t in range(8):
                            nc.tensor.matmul(ops_[:], attnT[t][:, cs],
                                             wo[t][:, mh * 512:(mh + 1) * 512],
                                             start=(t == 0), stop=(t == 7))
                        ost = tB.tile([128, 512], FP32, tag="ost")
                        nc.scalar.copy(ost[:], ops_[:])
                        nc.sync.dma_start(out_d[cs, mh * 512:(mh + 1) * 512], ost[:])

    nc.finalize()
    return nc


def _host_prep(cos, sin, W_qkv, W_o, proj):
    ratio = (NF ** -0.5) * (HD ** -0.5 + EPS_K)
    projS = np.ascontiguousarray((proj * ratio).T, dtype=np.float32)   # [d, f]
    projS2 = np.concatenate([projS, projS], axis=0)                    # [128, f]
    wqkT = np.ascontiguousarray(W_qkv[:1536].T, dtype=np.float32)      # [1024, 1536]
    wvT = np.ascontiguousarray(W_qkv[1536:].T, dtype=np.float32)       # [1024, 512]
    woT = np.ascontiguousarray(W_o.T, dtype=np.float32)                # [1024, 1024]
    sgn = np.concatenate([-np.ones(32, np.float32), np.ones(32, np.float32)])
    cosT1 = cos.T.astype(np.float32)                                   # [64, S]
    sinT1 = (sin.T * sgn[:, None]).astype(np.float32)
    cosT = np.concatenate([cosT1, cosT1], axis=0)                      # [128, S]
    sinTs = np.concatenate([sinT1, sinT1], axis=0)
    P = np.zeros((HD, HD), np.float32)
    for d in range(HD):
        P[(d + 32) % HD, d] = 1.0
    permP = np.zeros((128, 128), np.float32)
    permP[:64, :64] = P
    permP[64:, 64:] = P
    tri = np.triu(np.ones((CH, CH), np.float32))                       # keep j<=i
    ones128 = np.ones((128, 1), np.float32)
    onehot8 = np.zeros((128, 512), np.float32)
    for b0 in (0, 32, 64, 96):
        for g in range(8):
            onehot8[b0 + g, g * 64:(g + 1) * 64] = 1.0
    return dict(projS2=projS2, wqkT=wqkT, wvT=wvT, woT=woT, cosT=cosT,
                sinTs=sinTs, permP=permP, tri=tri, ones128=ones128,
                onehot8=onehot8)


_NC_CACHE = []


def kernel(**inputs):
    hidden = np.ascontiguousarray(np.asarray(inputs["hidden_states"], dtype=np.float32))
    cos = np.asarray(inputs["cos"], dtype=np.float32)
    sin = np.asarray(inputs["sin"], dtype=np.float32)
    W_qkv = np.asarray(inputs["W_qkv"], dtype=np.float32)
    W_o = np.asarray(inputs["W_o"], dtype=np.float32)
    proj = np.asarray(inputs["proj"], dtype=np.float32)

    prep = _host_prep(cos, sin, W_qkv, W_o, proj)
    shared = {k: prep[k] for k in ("wqkT", "wvT", "woT", "projS2", "permP",
                                   "tri", "ones128", "onehot8")}

    if not _NC_CACHE:
        _NC_CACHE.append(build_nc())
    nc = _NC_CACHE[0]

    in_maps = []
    for c in range(NCORES):
        b, rho = c // GROUP, c % GROUP
        sl = slice(rho * NPOS, (rho + 1) * NPOS)
        hT = np.ascontiguousarray(hidden[b, sl].T)
        wpref = np.zeros((128, GROUP), np.float32)
        wpref[:, :rho] = 1.0
        in_maps.append({"hT": hT,
                        "cosT": np.ascontiguousarray(prep["cosT"][:, sl]),
                        "sinTs": np.ascontiguousarray(prep["sinTs"][:, sl]),
                        "wpref": wpref, **shared})

    res = bass_utils.run_bass_kernel_spmd(nc, in_maps, core_ids=list(range(NCORES)))

    out = np.empty((B, S, HM), np.float32)
    for c in range(NCORES):
        b, rho = c // GROUP, c % GROUP
        out[b, rho * NPOS:(rho + 1) * NPOS, :] = res.results[c]["outp"]
    return out
